# revision 21
# baseline (speedup 1.0000x reference)
"""
DistanceSampling Trainium2 kernel (8 NeuronCores, SPMD over patch rows).

Computation per patch (2x2, stride 2) of x (1, 256, 512, 512):
  mean over the 4 patch elements (per channel), d_k = ||x_k - mean + eps||_2
  over channels, k* = argmax_k d_k (first occurrence), out = x_{k*}.
Output: (1, 256, 65536) fp32.

Design (per core: 64 image rows = 32 patch rows = 16 "qpair" groups, 8192 locs):
  - channels on SBUF partitions (2 blocks of 128), locations on free dim
  - D_k = 4*x_k - (sum_j x_j - 4*eps)  (argmax-equivalent scaling by 4),
    computed interleaved in one scalar_tensor_tensor per q (step-0 bcast of v)
  - dist_k = sum_c D_k^2 via TensorE one-hot weight columns: lhsT = E_k
    (128x4, ones in col k), rhs = square(D) strided view, accumulated over
    (k, cb) into one (4, 512) PSUM tile per qpair -> cheap LDWEIGHTS
  - (4,512) dist transposed back to location-major (128, 4) chunks on PE
  - tournament masks g1,g2,g3 on tiny tiles, transposed to free-major via
    M=1 PE transposes, broadcast to 128 partitions via K=1 matmul
  - selection: out = x0 (GpSimd copy), then 3 copy_predicated overwrites
    (exact first-occurrence argmax: strict > against running max)
"""

import sys

sys.path.insert(0, "/opt/trn_rl_repo")

import numpy as np

import concourse.bacc as bacc
import concourse.bass as bass
import concourse.mybir as mybir
import concourse.tile as tile
from concourse import masks
from concourse.bass_utils import run_bass_kernel_spmd

f32 = mybir.dt.float32
Alu = mybir.AluOpType
Act = mybir.ActivationFunctionType

EPS = 1e-6
C, H, W = 256, 512, 512
NCORES = 8
RPC = H // NCORES  # image rows per core (64)
QPC = RPC // 2  # patch rows per core (32)
QP = QPC // 2  # qpair groups per core (16)
FW = W // 2  # patches per row (256)
LPC = QPC * FW  # locations per core (8192)


def _kernel_body(tc):
    nc = tc.nc
    bf16 = mybir.dt.bfloat16
    x = nc.dram_tensor("x", [C, RPC, W], f32, kind="ExternalInput").ap()
    cA = nc.dram_tensor("cA", [4, 6], f32, kind="ExternalInput").ap()
    cM = nc.dram_tensor("cM", [6, 4], bf16, kind="ExternalInput").ap()
    cneed = nc.dram_tensor("cneed", [4, 1], f32, kind="ExternalInput").ap()
    cSEL = nc.dram_tensor("cSEL", [4, 384], bf16, kind="ExternalInput").ap()
    out = nc.dram_tensor("out", [C, LPC], f32, kind="ExternalOutput").ap()

    with (
        tc.tile_pool(name="const", bufs=1) as constp,
        tc.tile_pool(name="xin", bufs=7) as xp,
        tc.tile_pool(name="work", bufs=5) as wp,
        tc.tile_pool(name="small", bufs=6) as smp,
        tc.tile_pool(name="sel", bufs=4) as sp,
        tc.tile_pool(name="ps_dist", bufs=3, space=bass.MemorySpace.PSUM) as pd,
        tc.tile_pool(name="ps_dt", bufs=1, space=bass.MemorySpace.PSUM) as pdt,
        tc.tile_pool(name="ps_gt", bufs=1, space=bass.MemorySpace.PSUM) as pgt,
        tc.tile_pool(name="ps_mask", bufs=3, space=bass.MemorySpace.PSUM) as pm,
    ):
        # E[:, 4k+j] = 1 if j == k else 0 ; lhsT slice [:, 4k:4k+4] is the
        # one-hot weight block that routes sum_partitions(rhs) to PSUM row k
        E = constp.tile([128, 16], f32)
        nc.gpsimd.memset(E[:], 0.0)
        nc.gpsimd.memset(E[:, 0:16:5], 1.0)
        # small constant matrices for the mask pipeline (DMA'd from DRAM):
        # A (4,6): pairwise-difference weights; M (6,4): beats-count weights;
        # need (4,1): required beats-count per k; SEL (4,384): broadcast rows
        A = constp.tile([4, 6], f32)
        nc.sync.dma_start(A[:], cA)
        M = constp.tile([6, 4], bf16)
        nc.sync.dma_start(M[:], cM)
        need = constp.tile([4, 1], f32)
        nc.sync.dma_start(need[:], cneed)
        SEL = constp.tile([4, 384], bf16)
        nc.sync.dma_start(SEL[:], cSEL)

        for qp in range(QP):
            dist_ps = pd.tile([4, 512], f32, tag="dist_ps")  # [k, q'*256+f]
            Xs = []
            Ss = []
            for cb in range(2):
                X = xp.tile([128, 2048], f32, tag="X")
                nc.sync.dma_start(
                    X[:], x[cb * 128 : (cb + 1) * 128, 4 * qp : 4 * qp + 4, :]
                )
                Xs.append(X)
                xv = X[:].rearrange("p (a h f s) -> p a h f s", a=2, h=2, s=2)
                x0, x1 = xv[:, :, 0, :, 0], xv[:, :, 0, :, 1]  # (128, 2, 256)
                x2, x3 = xv[:, :, 1, :, 0], xv[:, :, 1, :, 1]

                p_t = wp.tile([128, 512], f32, tag="p")
                r_t = wp.tile([128, 512], f32, tag="r")
                pv = p_t[:].rearrange("p (a f) -> p a f", a=2)
                rv = r_t[:].rearrange("p (a f) -> p a f", a=2)
                nc.gpsimd.tensor_tensor(pv, x0, x1, Alu.add)
                nc.gpsimd.tensor_tensor(rv, x2, x3, Alu.add)
                # v = x0+x1+x2+x3 ; w = v/4 - eps  (D_k = x_k - w matches the
                # reference diff = x_k - mean + eps up to fp32 rounding order)
                v_t = wp.tile([128, 512], f32, tag="v")
                nc.vector.tensor_tensor(v_t[:], p_t[:], r_t[:], Alu.add)
                w_t = wp.tile([128, 512], f32, tag="w")
                nc.scalar.activation(
                    w_t[:], v_t[:], Act.Copy, bias=-EPS, scale=0.25
                )
                # D_k = x_k - w in k-major blocks: D[:, (a*4+k)*256 + f].
                # k0/k1 (row h=0) merged into one s-major op on GpSimd,
                # k2/k3 (row h=1) merged on VectorE.
                D = wp.tile([128, 2048], f32, tag="D")
                for a in range(2):
                    wrep = (
                        w_t[:, a * 256 : (a + 1) * 256]
                        .unsqueeze(1)
                        .broadcast_to([128, 2, 256])
                    )
                    for h, eng in ((0, nc.gpsimd), (1, nc.vector)):
                        src = X[
                            :, a * 1024 + h * 512 : a * 1024 + (h + 1) * 512
                        ].rearrange("p (f s) -> p s f", s=2)
                        dst = D[
                            :, (a * 4 + 2 * h) * 256 : (a * 4 + 2 * h + 2) * 256
                        ].rearrange("p (s f) -> p s f", f=256)
                        eng.tensor_tensor(dst, src, wrep, Alu.subtract)
                nc.scalar.activation(D[:], D[:], Act.Square)  # S = D^2 in place
                Ss.append(D)
                # dist rows: one-hot weight cols, accumulate over (cb, k)
                sv = D[:].rearrange("p (a k) -> p a k", a=2)
                for k in range(4):
                    nc.tensor.matmul(
                        dist_ps[:],
                        E[:, 4 * k : 4 * k + 4],
                        sv[:, :, k * 256 : (k + 1) * 256],
                        start=(cb == 0 and k == 0),
                        stop=(cb == 1 and k == 3),
                    )

            # pairwise diffs (6, 512) = A^T @ dist, sign, beats-count, one-hot
            dist4 = smp.tile([4, 512], f32, tag="dist4")
            nc.scalar.copy(dist4[:], dist_ps[:])
            diff_ps = pgt.tile([6, 512], f32, tag="diff_ps")
            nc.tensor.matmul(diff_ps[:], A[:], dist4[:])
            u_sb = smp.tile([6, 512], bf16, tag="u_sb")
            nc.vector.tensor_scalar(
                out=u_sb[:], in0=diff_ps[:], scalar1=0.0, scalar2=None, op0=Alu.is_gt
            )
            b_ps = pdt.tile([4, 512], f32, tag="b_ps")
            nc.tensor.matmul(b_ps[:], M[:], u_sb[:])
            m_sb = smp.tile([4, 512], bf16, tag="m_sb")
            nc.vector.tensor_scalar(
                out=m_sb[:], in0=b_ps[:], scalar1=need[:], scalar2=None, op0=Alu.is_equal
            )
            # broadcast m_1..m_3 to 128 partitions (K=4 matmuls, bf16)
            mask_list = []
            for g in range(3):
                mask_ps = pm.tile([128, 512], f32, tag="mask")
                nc.tensor.matmul(mask_ps[:], SEL[:, g * 128 : (g + 1) * 128], m_sb[:])
                mask_list.append(mask_ps)

            # selection: out = x0; overwrite with x1/x2/x3 where masks set.
            # mask location order (c, p) equals o_t order q'*256 + f, since
            # c = q'*2 + f//128 and p = f%128.
            for cb in range(2):
                xv = Xs[cb][:].rearrange("p (a h f s) -> p a h f s", a=2, h=2, s=2)
                o_t = sp.tile([128, 512], f32, tag="o")
                for a in range(2):
                    nc.scalar.copy(o_t[:, a * 256 : (a + 1) * 256], xv[:, a, 0, :, 0])
                for g, (hk, sk) in enumerate(((0, 1), (1, 0), (1, 1))):
                    mi = mask_list[g][:].bitcast(mybir.dt.int32)
                    for a in range(2):
                        nc.vector.copy_predicated(
                            o_t[:, a * 256 : (a + 1) * 256],
                            mi[:, a * 256 : (a + 1) * 256],
                            xv[:, a, hk, :, sk],
                        )
                nc.sync.dma_start(
                    out[cb * 128 : (cb + 1) * 128, qp * 512 : (qp + 1) * 512], o_t[:]
                )


def _const_arrays():
    import ml_dtypes

    A = np.zeros((4, 6), np.float32)
    for j, (ka, kb) in enumerate(((1, 0), (2, 0), (2, 1), (3, 0), (3, 1), (3, 2))):
        A[ka, j] = 1.0
        A[kb, j] = -1.0
    M = np.array(
        [
            [-1, 1, 0, 0],
            [-1, 0, 1, 0],
            [0, -1, 1, 0],
            [-1, 0, 0, 1],
            [0, -1, 0, 1],
            [0, 0, -1, 1],
        ],
        np.float32,
    ).astype(ml_dtypes.bfloat16)
    need = np.array([[0.0], [1.0], [2.0], [3.0]], np.float32)
    SEL = np.zeros((4, 384), np.float32)
    for g, k in enumerate((1, 2, 3)):
        SEL[k, g * 128 : (g + 1) * 128] = 1.0
    SEL = SEL.astype(ml_dtypes.bfloat16)
    return {"cA": A, "cM": M, "cneed": need, "cSEL": SEL}


_compiled_nc = None


def _get_compiled():
    global _compiled_nc
    if _compiled_nc is None:
        nc = bacc.Bacc(
            "TRN2", target_bir_lowering=False, debug=False, num_devices=NCORES
        )
        with tile.TileContext(nc) as tc:
            _kernel_body(tc)
        nc.compile()
        _compiled_nc = nc
    return _compiled_nc


def run_sharded(x_full: np.ndarray, **spmd_kwargs):
    """x_full: (1, C, H, W) fp32. Returns (results, raw) where results is the
    assembled (1, C, L) array and raw is the BassKernelResults."""
    nc = _get_compiled()
    xs = x_full[0]  # (C, H, W)
    consts = _const_arrays()
    in_maps = [
        {"x": np.ascontiguousarray(xs[:, m * RPC : (m + 1) * RPC, :]), **consts}
        for m in range(NCORES)
    ]
    raw = run_bass_kernel_spmd(nc, in_maps, list(range(NCORES)), **spmd_kwargs)
    outs = [raw.results[m]["out"] for m in range(NCORES)]  # (C, LPC) each
    full = np.concatenate(outs, axis=1)[None]  # (1, C, L)
    return full, raw


def kernel(x: np.ndarray) -> np.ndarray:
    x = np.asarray(x, dtype=np.float32)
    assert x.shape == (1, C, H, W), x.shape
    full, _ = run_sharded(x)
    return full


# revision 22
# speedup vs baseline: 1.0151x; 1.0151x over previous
"""
DistanceSampling Trainium2 kernel (8 NeuronCores, SPMD over patch rows).

Computation per patch (2x2, stride 2) of x (1, 256, 512, 512):
  mean over the 4 patch elements (per channel), d_k = ||x_k - mean + eps||_2
  over channels, k* = argmax_k d_k (first occurrence), out = x_{k*}.
Output: (1, 256, 65536) fp32.

Design (per core: 64 image rows = 32 patch rows = 16 "qpair" groups, 8192 locs):
  - channels on SBUF partitions (2 blocks of 128), locations on free dim
  - D_k = 4*x_k - (sum_j x_j - 4*eps)  (argmax-equivalent scaling by 4),
    computed interleaved in one scalar_tensor_tensor per q (step-0 bcast of v)
  - dist_k = sum_c D_k^2 via TensorE one-hot weight columns: lhsT = E_k
    (128x4, ones in col k), rhs = square(D) strided view, accumulated over
    (k, cb) into one (4, 512) PSUM tile per qpair -> cheap LDWEIGHTS
  - (4,512) dist transposed back to location-major (128, 4) chunks on PE
  - tournament masks g1,g2,g3 on tiny tiles, transposed to free-major via
    M=1 PE transposes, broadcast to 128 partitions via K=1 matmul
  - selection: out = x0 (GpSimd copy), then 3 copy_predicated overwrites
    (exact first-occurrence argmax: strict > against running max)
"""

import sys

sys.path.insert(0, "/opt/trn_rl_repo")

import numpy as np

import concourse.bacc as bacc
import concourse.bass as bass
import concourse.mybir as mybir
import concourse.tile as tile
from concourse import masks
from concourse.bass_utils import run_bass_kernel_spmd

f32 = mybir.dt.float32
Alu = mybir.AluOpType
Act = mybir.ActivationFunctionType

EPS = 1e-6
C, H, W = 256, 512, 512
NCORES = 8
RPC = H // NCORES  # image rows per core (64)
QPC = RPC // 2  # patch rows per core (32)
QP = QPC // 2  # qpair groups per core (16)
FW = W // 2  # patches per row (256)
LPC = QPC * FW  # locations per core (8192)


def _kernel_body(tc):
    nc = tc.nc
    bf16 = mybir.dt.bfloat16
    x = nc.dram_tensor("x", [C, RPC, W], f32, kind="ExternalInput").ap()
    cA = nc.dram_tensor("cA", [4, 6], f32, kind="ExternalInput").ap()
    cM = nc.dram_tensor("cM", [6, 4], bf16, kind="ExternalInput").ap()
    cneed = nc.dram_tensor("cneed", [4, 1], f32, kind="ExternalInput").ap()
    cSEL = nc.dram_tensor("cSEL", [4, 384], bf16, kind="ExternalInput").ap()
    out = nc.dram_tensor("out", [C, LPC], f32, kind="ExternalOutput").ap()

    with (
        tc.tile_pool(name="const", bufs=1) as constp,
        tc.tile_pool(name="xin", bufs=7) as xp,
        tc.tile_pool(name="work", bufs=5) as wp,
        tc.tile_pool(name="small", bufs=6) as smp,
        tc.tile_pool(name="sel", bufs=4) as sp,
        tc.tile_pool(name="ps_dist", bufs=3, space=bass.MemorySpace.PSUM) as pd,
        tc.tile_pool(name="ps_dt", bufs=1, space=bass.MemorySpace.PSUM) as pdt,
        tc.tile_pool(name="ps_gt", bufs=1, space=bass.MemorySpace.PSUM) as pgt,
        tc.tile_pool(name="ps_mask", bufs=3, space=bass.MemorySpace.PSUM) as pm,
    ):
        # E[:, 4k+j] = 1 if j == k else 0 ; lhsT slice [:, 4k:4k+4] is the
        # one-hot weight block that routes sum_partitions(rhs) to PSUM row k
        E = constp.tile([128, 16], f32)
        nc.gpsimd.memset(E[:], 0.0)
        nc.gpsimd.memset(E[:, 0:16:5], 1.0)
        # small constant matrices for the mask pipeline (DMA'd from DRAM):
        # A (4,6): pairwise-difference weights; M (6,4): beats-count weights;
        # need (4,1): required beats-count per k; SEL (4,384): broadcast rows
        A = constp.tile([4, 6], f32)
        nc.sync.dma_start(A[:], cA)
        M = constp.tile([6, 4], bf16)
        nc.sync.dma_start(M[:], cM)
        need = constp.tile([4, 1], f32)
        nc.sync.dma_start(need[:], cneed)
        SEL = constp.tile([4, 384], bf16)
        nc.sync.dma_start(SEL[:], cSEL)

        for qp in range(QP):
            dist_ps = pd.tile([4, 512], f32, tag="dist_ps")  # [k, q'*256+f]
            Xs = []
            Ss = []
            for cb in range(2):
                X = xp.tile([128, 2048], f32, tag="X")
                nc.sync.dma_start(
                    X[:], x[cb * 128 : (cb + 1) * 128, 4 * qp : 4 * qp + 4, :]
                )
                Xs.append(X)
                xv = X[:].rearrange("p (a h f s) -> p a h f s", a=2, h=2, s=2)
                x0, x1 = xv[:, :, 0, :, 0], xv[:, :, 0, :, 1]  # (128, 2, 256)
                x2, x3 = xv[:, :, 1, :, 0], xv[:, :, 1, :, 1]

                p_t = wp.tile([128, 512], f32, tag="p")
                r_t = wp.tile([128, 512], f32, tag="r")
                pv = p_t[:].rearrange("p (a f) -> p a f", a=2)
                rv = r_t[:].rearrange("p (a f) -> p a f", a=2)
                nc.gpsimd.tensor_tensor(pv, x0, x1, Alu.add)
                nc.gpsimd.tensor_tensor(rv, x2, x3, Alu.add)
                # v = x0+x1+x2+x3 ; w = v/4 - eps  (D_k = x_k - w matches the
                # reference diff = x_k - mean + eps up to fp32 rounding order)
                v_t = wp.tile([128, 512], f32, tag="v")
                nc.vector.tensor_tensor(v_t[:], p_t[:], r_t[:], Alu.add)
                w_t = wp.tile([128, 512], f32, tag="w")
                nc.scalar.activation(
                    w_t[:], v_t[:], Act.Copy, bias=-EPS, scale=0.25
                )
                # D_k = x_k - w in k-major blocks: D[:, (a*4+k)*256 + f].
                # k0/k1 (row h=0) merged into one s-major op on GpSimd,
                # k2/k3 (row h=1) merged on VectorE.
                # D block layout per a: [k0/k1 interleaved (512) | k2 (256) | k3 (256)]
                # h=0 on GpSimd reads X densely (interleaved out); h=1 on DVE
                D = wp.tile([128, 2048], f32, tag="D")
                for a in range(2):
                    wblk = w_t[:, a * 256 : (a + 1) * 256]
                    wrep = wblk.unsqueeze(2).broadcast_to([128, 256, 2])
                    src_e = X[:, a * 1024 : a * 1024 + 512].rearrange(
                        "p (f s) -> p f s", s=2
                    )
                    dst_e = D[:, a * 1024 : a * 1024 + 512].rearrange(
                        "p (f s) -> p f s", s=2
                    )
                    nc.gpsimd.tensor_tensor(dst_e, src_e, wrep, Alu.subtract)
                    src_o = X[
                        :, a * 1024 + 512 : a * 1024 + 1024
                    ].rearrange("p (f s) -> p s f", s=2)
                    dst_o = D[
                        :, a * 1024 + 512 : a * 1024 + 1024
                    ].rearrange("p (s f) -> p s f", f=256)
                    nc.vector.tensor_tensor(dst_o, src_o, wrep.transpose([0, 2, 1]), Alu.subtract)
                nc.scalar.activation(D[:], D[:], Act.Square)  # S = D^2 in place
                Ss.append(D)
                # dist rows: one-hot weight cols, accumulate over (cb, k)
                for k in range(4):
                    if k < 2:
                        rhs = D[:].rearrange("p (a j) -> p a j", a=2)[
                            :, :, k : 512 : 2
                        ]
                    else:
                        rhs = D[:].rearrange("p (a j) -> p a j", a=2)[
                            :, :, 512 + (k - 2) * 256 : 512 + (k - 1) * 256
                        ]
                    nc.tensor.matmul(
                        dist_ps[:],
                        E[:, 4 * k : 4 * k + 4],
                        rhs,
                        start=(cb == 0 and k == 0),
                        stop=(cb == 1 and k == 3),
                    )

            # pairwise diffs (6, 512) = A^T @ dist, sign, beats-count, one-hot
            dist4 = smp.tile([4, 512], f32, tag="dist4")
            nc.scalar.copy(dist4[:], dist_ps[:])
            diff_ps = pgt.tile([6, 512], f32, tag="diff_ps")
            nc.tensor.matmul(diff_ps[:], A[:], dist4[:])
            u_sb = smp.tile([6, 512], bf16, tag="u_sb")
            nc.vector.tensor_scalar(
                out=u_sb[:], in0=diff_ps[:], scalar1=0.0, scalar2=None, op0=Alu.is_gt
            )
            b_ps = pdt.tile([4, 512], f32, tag="b_ps")
            nc.tensor.matmul(b_ps[:], M[:], u_sb[:])
            m_sb = smp.tile([4, 512], bf16, tag="m_sb")
            nc.vector.tensor_scalar(
                out=m_sb[:], in0=b_ps[:], scalar1=need[:], scalar2=None, op0=Alu.is_equal
            )
            # broadcast m_1..m_3 to 128 partitions (K=4 matmuls, bf16)
            mask_list = []
            for g in range(3):
                mask_ps = pm.tile([128, 512], f32, tag="mask")
                nc.tensor.matmul(mask_ps[:], SEL[:, g * 128 : (g + 1) * 128], m_sb[:])
                mask_list.append(mask_ps)

            # selection: out = x0; overwrite with x1/x2/x3 where masks set.
            # mask location order (c, p) equals o_t order q'*256 + f, since
            # c = q'*2 + f//128 and p = f%128.
            for cb in range(2):
                xv = Xs[cb][:].rearrange("p (a h f s) -> p a h f s", a=2, h=2, s=2)
                o_t = sp.tile([128, 512], f32, tag="o")
                for a in range(2):
                    nc.scalar.copy(o_t[:, a * 256 : (a + 1) * 256], xv[:, a, 0, :, 0])
                for g, (hk, sk) in enumerate(((0, 1), (1, 0), (1, 1))):
                    mi = mask_list[g][:].bitcast(mybir.dt.int32)
                    for a in range(2):
                        nc.vector.copy_predicated(
                            o_t[:, a * 256 : (a + 1) * 256],
                            mi[:, a * 256 : (a + 1) * 256],
                            xv[:, a, hk, :, sk],
                        )
                nc.sync.dma_start(
                    out[cb * 128 : (cb + 1) * 128, qp * 512 : (qp + 1) * 512], o_t[:]
                )


def _const_arrays():
    import ml_dtypes

    A = np.zeros((4, 6), np.float32)
    for j, (ka, kb) in enumerate(((1, 0), (2, 0), (2, 1), (3, 0), (3, 1), (3, 2))):
        A[ka, j] = 1.0
        A[kb, j] = -1.0
    M = np.array(
        [
            [-1, 1, 0, 0],
            [-1, 0, 1, 0],
            [0, -1, 1, 0],
            [-1, 0, 0, 1],
            [0, -1, 0, 1],
            [0, 0, -1, 1],
        ],
        np.float32,
    ).astype(ml_dtypes.bfloat16)
    need = np.array([[0.0], [1.0], [2.0], [3.0]], np.float32)
    SEL = np.zeros((4, 384), np.float32)
    for g, k in enumerate((1, 2, 3)):
        SEL[k, g * 128 : (g + 1) * 128] = 1.0
    SEL = SEL.astype(ml_dtypes.bfloat16)
    return {"cA": A, "cM": M, "cneed": need, "cSEL": SEL}


_compiled_nc = None


def _get_compiled():
    global _compiled_nc
    if _compiled_nc is None:
        nc = bacc.Bacc(
            "TRN2", target_bir_lowering=False, debug=False, num_devices=NCORES
        )
        with tile.TileContext(nc) as tc:
            _kernel_body(tc)
        nc.compile()
        _compiled_nc = nc
    return _compiled_nc


def run_sharded(x_full: np.ndarray, **spmd_kwargs):
    """x_full: (1, C, H, W) fp32. Returns (results, raw) where results is the
    assembled (1, C, L) array and raw is the BassKernelResults."""
    nc = _get_compiled()
    xs = x_full[0]  # (C, H, W)
    consts = _const_arrays()
    in_maps = [
        {"x": np.ascontiguousarray(xs[:, m * RPC : (m + 1) * RPC, :]), **consts}
        for m in range(NCORES)
    ]
    raw = run_bass_kernel_spmd(nc, in_maps, list(range(NCORES)), **spmd_kwargs)
    outs = [raw.results[m]["out"] for m in range(NCORES)]  # (C, LPC) each
    full = np.concatenate(outs, axis=1)[None]  # (1, C, L)
    return full, raw


def kernel(x: np.ndarray) -> np.ndarray:
    x = np.asarray(x, dtype=np.float32)
    assert x.shape == (1, C, H, W), x.shape
    full, _ = run_sharded(x)
    return full
